# revision 17
# baseline (speedup 1.0000x reference)
"""Multi-head self-attention on 8 Trainium2 NeuronCores.

Problem: B=4, S=2048, D=1024, H=16 heads (dk=64), torch-Linear style
projections (y = x @ W.T + b), softmax attention, output projection.

Sharding: 8 cores = 4 batches x 2 head-groups (8 heads each).  Each core
computes, for its (batch b, group g):
    QT = (Wq_g/sqrt(dk)) @ x_b.T + bq_g/sqrt(dk)   [512, S]  (features on partitions)
    KT = Wk_g @ x_b.T                              [512, S]  (bk dropped: it only
                                                    shifts scores uniformly per
                                                    query and cancels in softmax)
    V  = x_b @ Wv_g.T + bv_g                       [S, 512]  (keys on partitions)
    per head pair pr (even head h0, odd h1), query stripe of 512, key block kb:
      sp[kb] = [scoresT_h0 | scoresT_h1]  (one 2-bank PSUM tile [128, 1024])
      ep[kb] = exp(sp[kb])  -- computed on ScalarE (exact spline exp) for 10 of
               16 key blocks and on VectorE for 6 of 16 via a one-instruction
               Schraudolph approximation: int16(round(s*184.665 + B)) bit-cast
               to bf16 == 2^(s*log2 e) with ~2% rms error.  Splitting exp
               across both engines keeps the TensorE (the bottleneck) saturated
               so the HAM clock gate stays at 2.4 GHz.
      pvb accumulates [V_h0|ones]^T ep_h0 and [ones|V_h1]^T ep_h1, yielding
      outputs and softmax denominators (ones-columns trick).
    normalization: reciprocal of the denominators (DVE approx), partition-swap
    of the two heads' reciprocals via SBUF->SBUF DMA, two tensor-tensor mults.
    partialT = Wo_g @ onorm     [1024, S]
Host sums the two group partials per batch, transposes, and adds bo.

Device dtypes: bf16 matmul operands, f32 PSUM; exp split ScalarE/VectorE.
"""

import math

import numpy as np
import ml_dtypes

import concourse.bass as bass
import concourse.bacc as bacc_mod
import concourse.mybir as mybir
import concourse.tile as tile
from concourse.bass_utils import run_bass_kernel_spmd

BF16 = mybir.dt.bfloat16
F32 = mybir.dt.float32
I16 = mybir.dt.int16
AF = mybir.ActivationFunctionType
MUL = mybir.AluOpType.mult
ADD = mybir.AluOpType.add

B, S, D, H = 4, 2048, 1024, 16
DK = D // H  # 64
NCORES = 8
GROUPS = 2  # tensor-parallel head groups
DG = D // GROUPS  # 512 features per group
P = 128
FT = DG // P  # 4 feature tiles per group == head pairs

# Schraudolph exp constants (bf16 bit trick): bits = s * A + BSCH, viewed bf16.
SCHRAUD_A = 128.0 / math.log(2.0)
SCHRAUD_B = 16256.0 - 7.25

# key blocks whose exp runs on VectorE (the rest go to ScalarE)
DVE_KBS = frozenset((1, 3, 5, 7, 9, 11, 13))


def build_attention_nc(seq: int = S, dbg: bool = False) -> bass.Bass:
    KB = seq // P  # 16 key blocks
    DKB = D // P  # 8 contraction blocks for projections
    QH = min(512, seq)  # query stripe == PSUM bank width in f32
    NQH = seq // QH
    QC = min(512, QH)
    NQC = seq // QC
    DT = D // P

    nc = bacc_mod.Bacc("TRN2", num_devices=NCORES)
    xt_d = nc.declare_dram_parameter("xt", [D, seq], BF16, isOutput=False)
    wqt_d = nc.declare_dram_parameter("wqt", [D, DG], BF16, isOutput=False)
    wkt_d = nc.declare_dram_parameter("wkt", [D, DG], BF16, isOutput=False)
    wvt_d = nc.declare_dram_parameter("wvt", [D, DG], BF16, isOutput=False)
    wot_d = nc.declare_dram_parameter("wot", [DG, D], BF16, isOutput=False)
    bq_d = nc.declare_dram_parameter("bqs", [P, FT], F32, isOutput=False)
    bv_d = nc.declare_dram_parameter("bvr", [1, DG], BF16, isOutput=False)
    out_d = nc.declare_dram_parameter("out", [D, seq], BF16, isOutput=True)
    if dbg:
        dq_d = nc.declare_dram_parameter("dq", [DG, seq], F32, isOutput=True)
        dk_d = nc.declare_dram_parameter("dk", [DG, seq], F32, isOutput=True)
        dv_d = nc.declare_dram_parameter("dv", [P * KB, 2 * DG], F32, isOutput=True)
        de_d = nc.declare_dram_parameter("de", [P, 4 * 512], F32, isOutput=True)
        dpv_d = nc.declare_dram_parameter("dpv", [P, 2 * 512], F32, isOutput=True)
        dr_d = nc.declare_dram_parameter("dr", [P, 2 * 512], F32, isOutput=True)
        dn_d = nc.declare_dram_parameter("dn", [DG, seq], F32, isOutput=True)

    with tile.TileContext(nc) as tc:
        with (
            tc.tile_pool(name="persist", bufs=1) as persist,
            tc.tile_pool(name="sp_ps", bufs=2, space="PSUM") as sp_ps,
            tc.tile_pool(name="pv_ps", bufs=2, space="PSUM") as pv_ps,
            tc.tile_pool(name="epool", bufs=6) as e_pool,
            tc.tile_pool(name="mpool", bufs=2) as m_pool,
        ):
            ones_bf = persist.tile([1, P], BF16, name="ones_bf")
            nc.vector.memset(ones_bf, 1.0)
            bq_sb = persist.tile([P, FT], F32, name="bq_sb")
            nc.scalar.dma_start(bq_sb, bq_d[:, :])
            bv_sb = persist.tile([1, DG], BF16, name="bv_sb")
            nc.scalar.dma_start(bv_sb, bv_d[:, :])

            # input DMAs, ordered so QT can start as early as possible
            xt_sb = []
            wqt_sb = []
            wkt_sb = []
            wvt_sb = []
            for i in range(DKB):
                wq_i = persist.tile([P, DG], BF16, name=f"wqts{i}")
                nc.scalar.dma_start(wq_i, wqt_d[i * P : (i + 1) * P, :])
                wqt_sb.append(wq_i)
                xti = persist.tile([P, seq], BF16, name=f"xts{i}")
                # first query-column chunk of every k-block lands first so the
                # QT matmuls can start ~4x earlier
                nc.sync.dma_start(
                    xti[:, 0:QC], xt_d[i * P : (i + 1) * P, 0:QC]
                )
                xt_sb.append(xti)
            for c in range(1, NQC):
                csl = slice(c * QC, (c + 1) * QC)
                for i in range(DKB):
                    nc.sync.dma_start(
                        xt_sb[i][:, csl], xt_d[i * P : (i + 1) * P, csl]
                    )
            for i in range(DKB):
                wk_i = persist.tile([P, DG], BF16, name=f"wkts{i}")
                nc.sync.dma_start(wk_i, wkt_d[i * P : (i + 1) * P, :])
                wkt_sb.append(wk_i)
            for i in range(DKB):
                wv_i = persist.tile([P, DG], BF16, name=f"wvts{i}")
                nc.sync.dma_start(wv_i, wvt_d[i * P : (i + 1) * P, :])
                wvt_sb.append(wv_i)
            wot_sb = []
            for ft in range(FT):
                w_o = persist.tile([P, D], BF16, name=f"wot{ft}")
                nc.sync.dma_start(w_o, wot_d[ft * P : (ft + 1) * P, :])
                wot_sb.append(w_o)

            # zero-padded per-head Q tiles: qt0 = [Q_h0; 0], qt1 = [0; Q_h1].
            # Score matmuls can then use the full 128-row KT stationary (the
            # zero half contributes nothing), keeping every attention LDWEIGHTS
            # uniform 128-row so the PE hides them behind running matmuls.
            qt0_sb = [persist.tile([P, seq], BF16, name=f"qt0_{i}") for i in range(FT)]
            qt1_sb = [persist.tile([P, seq], BF16, name=f"qt1_{i}") for i in range(FT)]
            for ft in range(FT):
                nc.vector.memset(qt0_sb[ft][64:128, :], 0.0)
                nc.vector.memset(qt1_sb[ft][0:64, :], 0.0)
            kt_sb = [persist.tile([P, seq], BF16, name=f"kt{i}") for i in range(FT)]
            # v2 holds, per 128-col head block: even heads [V_h | ones],
            # odd heads [ones | V_h] -- the ones columns make the PV matmul
            # also produce the softmax denominators on the other 64 rows.
            v2_sb = [persist.tile([P, 2 * DG], BF16, name=f"v{i}") for i in range(KB)]
            onorm = [persist.tile([P, seq], BF16, name=f"onorm{i}") for i in range(FT)]

            # ---------------- phase 1: projections ----------------
            # QT (bias via ACT), then KT, then V -- keeps the PE streaming.
            for c in range(NQC):
                csl = slice(c * QC, (c + 1) * QC)
                for ft in range(FT):
                    fsl = slice(ft * P, (ft + 1) * P)
                    psq = sp_ps.tile([P, QC], F32, name="psq", tag="sp")
                    for k in range(DKB):
                        nc.tensor.matmul(
                            psq,
                            lhsT=wqt_sb[k][:, fsl],
                            rhs=xt_sb[k][:, csl],
                            start=k == 0,
                            stop=k == DKB - 1,
                        )
                    nc.scalar.activation(
                        qt0_sb[ft][0:64, csl], psq[0:64, :], AF.Identity,
                        bias=bq_sb[0:64, ft : ft + 1],
                    )
                    nc.scalar.activation(
                        qt1_sb[ft][64:128, csl], psq[64:128, :], AF.Identity,
                        bias=bq_sb[64:128, ft : ft + 1],
                    )
            for ft in range(FT):
                fsl = slice(ft * P, (ft + 1) * P)
                for c in range(NQC):
                    csl = slice(c * QC, (c + 1) * QC)
                    psk = sp_ps.tile([P, QC], F32, name="psk", tag="sp")
                    for k in range(DKB):
                        nc.tensor.matmul(
                            psk,
                            lhsT=wkt_sb[k][:, fsl],
                            rhs=xt_sb[k][:, csl],
                            start=k == 0,
                            stop=k == DKB - 1,
                        )
                    nc.vector.tensor_copy(kt_sb[ft][:, csl], psk)

            # V: keys on partitions, features on free dim (+ bias via K=1 matmul)
            for kb in range(KB):
                ksl = slice(kb * P, (kb + 1) * P)
                psv = pv_ps.tile([P, DG], F32, name="psv", tag="pvb")
                for k in range(DKB):
                    nc.tensor.matmul(
                        psv,
                        lhsT=xt_sb[k][:, ksl],
                        rhs=wvt_sb[k],
                        start=k == 0,
                        stop=False,
                    )
                nc.tensor.matmul(
                    psv, lhsT=ones_bf, rhs=bv_sb, start=False, stop=True
                )
                nc.gpsimd.memset(v2_sb[kb], 1.0)
                # even heads -> cols [256q, 0:64); odd heads -> [256q+192, 256q+256)
                nc.vector.tensor_copy(
                    v2_sb[kb].rearrange("p (q c) -> p q c", c=256)[:, :, 0:64],
                    psv.rearrange("p (q c) -> p q c", c=128)[:, :, 0:64],
                )
                nc.vector.tensor_copy(
                    v2_sb[kb].rearrange("p (q c) -> p q c", c=256)[:, :, 192:256],
                    psv.rearrange("p (q c) -> p q c", c=128)[:, :, 64:128],
                )

            # ---------------- phase 2: attention ----------------
            # Software-pipelined: scores kb | exp kb-1 | PV kb-3, so the PE
            # queue never drains and the exp engines run one step behind with
            # ~1.7us of slack.  The previous stripe's normalization ops are
            # spread across this stripe's steps (keyed by step number) so the
            # DVE never sees a burst that would starve the PE of exp tiles.
            PVLAG = 4
            pending_norm: dict = {}

            def run_pending(step):
                for fn in pending_norm.pop(step, ()):  # noqa: B023
                    fn()

            def flush_norm():
                for st in sorted(pending_norm):
                    for fn in pending_norm[st]:
                        fn()
                pending_norm.clear()

            for pr in range(FT):
                h0c = slice((2 * pr) * P, (2 * pr + 1) * P)
                h1c = slice((2 * pr + 1) * P, (2 * pr + 2) * P)
                for qh in range(NQH):
                    qsl = slice(qh * QH, (qh + 1) * QH)
                    pvb = pv_ps.tile([P, 2 * QH], F32, name="pvb", tag="pvb")
                    sp_l = [None] * KB
                    ep_l = [None] * KB
                    for step in range(KB + PVLAG):
                        kb2 = step - PVLAG
                        if step < KB:
                            kb = step
                            ksl = slice(kb * P, (kb + 1) * P)
                            sp = sp_ps.tile([P, 2 * QH], F32, name="sp", tag="sp")
                            sp_l[kb] = sp
                            nc.tensor.matmul(
                                sp[:, 0:QH],
                                lhsT=kt_sb[pr][:, ksl],
                                rhs=qt0_sb[pr][:, qsl],
                                start=True, stop=True,
                            )
                            if kb2 >= 0:
                                nc.tensor.matmul(
                                    pvb[:, 0:QH], lhsT=v2_sb[kb2][:, h0c],
                                    rhs=ep_l[kb2][:, 0:QH],
                                    start=kb2 == 0, stop=kb2 == KB - 1,
                                )
                            nc.tensor.matmul(
                                sp[:, QH : 2 * QH],
                                lhsT=kt_sb[pr][:, ksl],
                                rhs=qt1_sb[pr][:, qsl],
                                start=True, stop=True,
                            )
                            if kb2 >= 0:
                                nc.tensor.matmul(
                                    pvb[:, QH : 2 * QH], lhsT=v2_sb[kb2][:, h1c],
                                    rhs=ep_l[kb2][:, QH : 2 * QH],
                                    start=kb2 == 0, stop=kb2 == KB - 1,
                                )
                        elif kb2 >= 0:
                            nc.tensor.matmul(
                                pvb[:, 0:QH], lhsT=v2_sb[kb2][:, h0c],
                                rhs=ep_l[kb2][:, 0:QH],
                                start=kb2 == 0, stop=kb2 == KB - 1,
                            )
                            nc.tensor.matmul(
                                pvb[:, QH : 2 * QH], lhsT=v2_sb[kb2][:, h1c],
                                rhs=ep_l[kb2][:, QH : 2 * QH],
                                start=kb2 == 0, stop=kb2 == KB - 1,
                            )
                        if 1 <= step <= KB:
                            kb1 = step - 1
                            ep = e_pool.tile([P, 2 * QH], BF16, name="ep", tag="ep")
                            ep_l[kb1] = ep
                            if kb1 in DVE_KBS:
                                nc.vector.tensor_scalar(
                                    ep.bitcast(I16), sp_l[kb1],
                                    SCHRAUD_A, SCHRAUD_B, MUL, ADD,
                                )
                            else:
                                nc.scalar.activation(ep, sp_l[kb1], AF.Exp)
                            if dbg and pr == 0 and qh == 0 and kb1 in (0, 1):
                                dtmp = m_pool.tile(
                                    [P, 2 * QH], F32, name="dtmp", tag="dtmp"
                                )
                                nc.vector.tensor_copy(dtmp, ep)
                                nc.sync.dma_start(
                                    de_d[:, kb1 * 2 * QH : (kb1 + 1) * 2 * QH], dtmp
                                )
                        run_pending(step)

                    # normalization for this stripe, scheduled into the next
                    # stripe's steps.  reciprocal_approx_fast (custom DVE op)
                    # misbehaves at base_partition 64, so den_h0 (rows 64:128)
                    # is copied to SBUF and DMA-shifted down to rows 0:64
                    # before its reciprocal; den_h1 is already at rows 0:64.
                    ctmp = m_pool.tile([P, QH], F32, name="ctmp", tag="ctmp")
                    rec = m_pool.tile([P, QH], F32, name="rec", tag="rec")
                    dsw = m_pool.tile([P, QH], F32, name="dsw", tag="dsw")
                    rsw = m_pool.tile([P, QH], F32, name="rsw", tag="rsw")

                    def n_copy(pvb=pvb, ctmp=ctmp, rec=rec, dsw=dsw, rsw=rsw):
                        nc.scalar.copy(ctmp[64:128, :], pvb[64:128, 0:QH])
                        nc.sync.dma_start(dsw[0:64, :], ctmp[64:128, :])

                    def n_rec(pvb=pvb, rec=rec, rsw=rsw):
                        nc.vector.reciprocal_approx_fast(
                            rec[0:64, :], pvb[0:64, QH : 2 * QH]
                        )
                        nc.sync.dma_start(rsw[64:128, :], rec[0:64, :])

                    def n_rsw(dsw=dsw, rsw=rsw):
                        nc.vector.reciprocal_approx_fast(rsw[0:64, :], dsw[0:64, :])

                    def n_mul0(pr=pr, qsl=qsl, pvb=pvb, rsw=rsw):
                        nc.vector.tensor_tensor(
                            onorm[pr][0:64, qsl], pvb[0:64, 0:QH],
                            rsw[0:64, :], MUL,
                        )

                    def n_mul1(pr=pr, qsl=qsl, pvb=pvb, rsw=rsw):
                        nc.vector.tensor_tensor(
                            onorm[pr][64:128, qsl], pvb[64:128, QH : 2 * QH],
                            rsw[64:128, :], MUL,
                        )

                    pending_norm = {2: [n_copy], 3: [n_rec], 7: [n_rsw],
                                    9: [n_mul0], 11: [n_mul1]}
                    if dbg and pr == 0 and qh == 0:
                        dtmp2 = m_pool.tile([P, 2 * QH], F32, name="dtmp2", tag="dtmp")
                        nc.vector.tensor_copy(dtmp2, pvb)
                        nc.sync.dma_start(dpv_d[:, :], dtmp2)

            flush_norm()

            if dbg:
                with tc.tile_pool(name="dbgp", bufs=2) as dbg_pool:
                    for ft in range(FT):
                        fs = slice(ft * P, (ft + 1) * P)
                        dqs = dbg_pool.tile([P, seq], F32, name="dqs", tag="dbg")
                        nc.vector.tensor_copy(dqs[0:64, :], qt0_sb[ft][0:64, :])
                        nc.vector.tensor_copy(dqs[64:128, :], qt1_sb[ft][64:128, :])
                        nc.sync.dma_start(dq_d[fs, :], dqs)
                        dks = dbg_pool.tile([P, seq], F32, name="dks", tag="dbg")
                        nc.vector.tensor_copy(dks, kt_sb[ft])
                        nc.sync.dma_start(dk_d[fs, :], dks)
                        dns = dbg_pool.tile([P, seq], F32, name="dns", tag="dbg")
                        nc.vector.tensor_copy(dns, onorm[ft])
                        nc.sync.dma_start(dn_d[fs, :], dns)
                    for kb in range(KB):
                        ks = slice(kb * P, (kb + 1) * P)
                        dvs = dbg_pool.tile([P, 2 * DG], F32, name="dvs", tag="dbg")
                        nc.vector.tensor_copy(dvs, v2_sb[kb])
                        nc.sync.dma_start(dv_d[ks, :], dvs)

            # ---------------- phase 3: output projection ----------------
            with tc.tile_pool(name="osb", bufs=4) as o_sb_pool:
                for dt in range(DT):
                    dsl = slice(dt * P, (dt + 1) * P)
                    for c in range(NQC):
                        csl = slice(c * QC, (c + 1) * QC)
                        pso = sp_ps.tile([P, QC], F32, name="pso", tag="sp")
                        for ft in range(FT):
                            nc.tensor.matmul(
                                pso,
                                lhsT=wot_sb[ft][:, dsl],
                                rhs=onorm[ft][:, csl],
                                start=ft == 0,
                                stop=ft == FT - 1,
                            )
                        o_sb = o_sb_pool.tile([P, QC], BF16, name="o_sb", tag="osb")
                        if (dt * NQC + c) % 2 == 0:
                            nc.vector.tensor_copy(o_sb, pso)
                        else:
                            nc.scalar.copy(o_sb, pso)
                        nc.sync.dma_start(out_d[dsl, csl], o_sb)

    return nc


_CACHE: dict = {}


def _get_nc(seq: int = S) -> bass.Bass:
    key = f"nc{seq}"
    if key not in _CACHE:
        nc = build_attention_nc(seq)
        nc.finalize()  # runs Bacc.compile(): reg alloc + wait legalization
        _CACHE[key] = nc
    return _CACHE[key]


def make_in_maps(x, Wq, bq, Wk, Wv, bv, Wo, seq: int = S):
    bf = ml_dtypes.bfloat16
    scale = 1.0 / math.sqrt(DK)
    x = np.asarray(x, np.float32)
    Wq = np.asarray(Wq, np.float32)
    bq = np.asarray(bq, np.float32)
    Wk = np.asarray(Wk, np.float32)
    Wv = np.asarray(Wv, np.float32)
    bv = np.asarray(bv, np.float32)
    Wo = np.asarray(Wo, np.float32)
    in_maps = []
    for core in range(NCORES):
        b, g = divmod(core, GROUPS)
        gsl = slice(g * DG, (g + 1) * DG)
        in_maps.append(
            {
                "xt": np.ascontiguousarray(x[b, :seq, :].T).astype(bf),
                "wqt": np.ascontiguousarray((Wq[gsl, :] * scale).T).astype(bf),
                "wkt": np.ascontiguousarray(Wk[gsl, :].T).astype(bf),
                "wvt": np.ascontiguousarray(Wv[gsl, :].T).astype(bf),
                "wot": np.ascontiguousarray(Wo[:, gsl].T).astype(bf),
                "bqs": np.ascontiguousarray(
                    (bq[gsl] * scale).astype(np.float32).reshape(FT, P).T
                ),
                "bvr": bv[gsl].astype(bf).reshape(1, DG),
            }
        )
    return in_maps


def run_device(in_maps, seq: int = S, trace: bool = False):
    nc = _get_nc(seq)
    return run_bass_kernel_spmd(nc, in_maps, list(range(NCORES)), trace=trace)


def kernel(x, Wq, bq, Wk, bk, Wv, bv, Wo, bo):
    in_maps = make_in_maps(x, Wq, bq, Wk, Wv, bv, Wo)
    res = run_device(in_maps).results
    bo = np.asarray(bo, np.float32)
    out = np.empty((B, S, D), np.float32)
    for b in range(B):
        acc = res[2 * b]["out"].astype(np.float32) + res[2 * b + 1]["out"].astype(
            np.float32
        )
        out[b] = acc.T + bo[None, :]
    return out


# revision 18
# speedup vs baseline: 1.2211x; 1.2211x over previous
"""Multi-head self-attention on 8 Trainium2 NeuronCores.

Problem: B=4, S=2048, D=1024, H=16 heads (dk=64), torch-Linear style
projections (y = x @ W.T + b), softmax attention, output projection.

Sharding: 8 cores = 4 batches x 2 head-groups (8 heads each).  Each core
computes, for its (batch b, group g):
    QT = (Wq_g/sqrt(dk)) @ x_b.T + bq_g/sqrt(dk)   [512, S]  (features on partitions)
    KT = Wk_g @ x_b.T                              [512, S]  (bk dropped: it only
                                                    shifts scores uniformly per
                                                    query and cancels in softmax)
    V  = x_b @ Wv_g.T + bv_g                       [S, 512]  (keys on partitions)
    per head pair pr (even head h0, odd h1), query stripe of 512, key block kb:
      sp[kb] = [scoresT_h0 | scoresT_h1]  (one 2-bank PSUM tile [128, 1024])
      ep[kb] = exp(sp[kb])  -- computed on ScalarE (exact spline exp) for 10 of
               16 key blocks and on VectorE for 6 of 16 via a one-instruction
               Schraudolph approximation: int16(round(s*184.665 + B)) bit-cast
               to bf16 == 2^(s*log2 e) with ~2% rms error.  Splitting exp
               across both engines keeps the TensorE (the bottleneck) saturated
               so the HAM clock gate stays at 2.4 GHz.
      pvb accumulates [V_h0|ones]^T ep_h0 and [ones|V_h1]^T ep_h1, yielding
      outputs and softmax denominators (ones-columns trick).
    normalization: reciprocal of the denominators (DVE approx), partition-swap
    of the two heads' reciprocals via SBUF->SBUF DMA, two tensor-tensor mults.
    partialT = Wo_g @ onorm     [1024, S]
Host sums the two group partials per batch, transposes, and adds bo.

Device dtypes: bf16 matmul operands, f32 PSUM; exp split ScalarE/VectorE.
"""

import math

import numpy as np
import ml_dtypes

import concourse.bass as bass
import concourse.bacc as bacc_mod
import concourse.mybir as mybir
import concourse.tile as tile
from concourse.bass_utils import run_bass_kernel_spmd

BF16 = mybir.dt.bfloat16
F32 = mybir.dt.float32
I16 = mybir.dt.int16
AF = mybir.ActivationFunctionType
MUL = mybir.AluOpType.mult
ADD = mybir.AluOpType.add

B, S, D, H = 4, 2048, 1024, 16
DK = D // H  # 64
NCORES = 8
GROUPS = 2  # tensor-parallel head groups
DG = D // GROUPS  # 512 features per group
P = 128
FT = DG // P  # 4 feature tiles per group == head pairs

# Schraudolph exp constants (bf16 bit trick): bits = s * A + BSCH, viewed bf16.
SCHRAUD_A = 128.0 / math.log(2.0)
SCHRAUD_B = 16256.0 - 7.25

# key blocks whose exp runs on VectorE (the rest go to ScalarE)
DVE_KBS = frozenset((1, 3, 5, 7, 9, 11, 13))


def build_attention_nc(seq: int = S, dbg: bool = False) -> bass.Bass:
    KB = seq // P  # 16 key blocks
    DKB = D // P  # 8 contraction blocks for projections
    QH = min(512, seq)  # query stripe == PSUM bank width in f32
    NQH = seq // QH
    QC = min(512, QH)
    NQC = seq // QC
    DT = D // P

    nc = bacc_mod.Bacc("TRN2", num_devices=NCORES)
    xt_d = nc.declare_dram_parameter("xt", [D, seq], BF16, isOutput=False)
    wqt_d = nc.declare_dram_parameter("wqt", [D, DG], BF16, isOutput=False)
    wkt_d = nc.declare_dram_parameter("wkt", [D, DG], BF16, isOutput=False)
    wvt_d = nc.declare_dram_parameter("wvt", [D, DG], BF16, isOutput=False)
    wot_d = nc.declare_dram_parameter("wot", [DG, D], BF16, isOutput=False)
    bq_d = nc.declare_dram_parameter("bqs", [P, FT], F32, isOutput=False)
    bv_d = nc.declare_dram_parameter("bvr", [1, DG], BF16, isOutput=False)
    out_d = nc.declare_dram_parameter("out", [D, seq], BF16, isOutput=True)
    if dbg:
        dq_d = nc.declare_dram_parameter("dq", [DG, seq], F32, isOutput=True)
        dk_d = nc.declare_dram_parameter("dk", [DG, seq], F32, isOutput=True)
        dv_d = nc.declare_dram_parameter("dv", [P * KB, 2 * DG], F32, isOutput=True)
        de_d = nc.declare_dram_parameter("de", [P, 4 * 512], F32, isOutput=True)
        dpv_d = nc.declare_dram_parameter("dpv", [P, 2 * 512], F32, isOutput=True)
        dr_d = nc.declare_dram_parameter("dr", [P, 2 * 512], F32, isOutput=True)
        dn_d = nc.declare_dram_parameter("dn", [DG, seq], F32, isOutput=True)

    with tile.TileContext(nc) as tc:
        with (
            tc.tile_pool(name="persist", bufs=1) as persist,
            tc.tile_pool(name="sp_ps", bufs=3, space="PSUM") as sp_ps,
            tc.tile_pool(name="pv_ps", bufs=1, space="PSUM") as pv_ps,
            tc.tile_pool(name="epool", bufs=6) as e_pool,
            tc.tile_pool(name="mpool", bufs=2) as m_pool,
        ):
            ones_bf = persist.tile([1, P], BF16, name="ones_bf")
            nc.vector.memset(ones_bf, 1.0)
            bq_sb = persist.tile([P, FT], F32, name="bq_sb")
            nc.scalar.dma_start(bq_sb, bq_d[:, :])
            bv_sb = persist.tile([1, DG], BF16, name="bv_sb")
            nc.scalar.dma_start(bv_sb, bv_d[:, :])

            # input DMAs, ordered so QT can start as early as possible
            xt_sb = []
            wqt_sb = []
            wkt_sb = []
            wvt_sb = []
            for i in range(DKB):
                wq_i = persist.tile([P, DG], BF16, name=f"wqts{i}")
                nc.scalar.dma_start(wq_i, wqt_d[i * P : (i + 1) * P, :])
                wqt_sb.append(wq_i)
                xti = persist.tile([P, seq], BF16, name=f"xts{i}")
                # first query-column chunk of every k-block lands first so the
                # QT matmuls can start ~4x earlier
                nc.sync.dma_start(
                    xti[:, 0:QC], xt_d[i * P : (i + 1) * P, 0:QC]
                )
                xt_sb.append(xti)
            for c in range(1, NQC):
                csl = slice(c * QC, (c + 1) * QC)
                for i in range(DKB):
                    nc.sync.dma_start(
                        xt_sb[i][:, csl], xt_d[i * P : (i + 1) * P, csl]
                    )
            for i in range(DKB):
                wk_i = persist.tile([P, DG], BF16, name=f"wkts{i}")
                nc.sync.dma_start(wk_i, wkt_d[i * P : (i + 1) * P, :])
                wkt_sb.append(wk_i)
            for i in range(DKB):
                wv_i = persist.tile([P, DG], BF16, name=f"wvts{i}")
                nc.sync.dma_start(wv_i, wvt_d[i * P : (i + 1) * P, :])
                wvt_sb.append(wv_i)
            wot_sb = []
            for ft in range(FT):
                w_o = persist.tile([P, D], BF16, name=f"wot{ft}")
                nc.sync.dma_start(w_o, wot_d[ft * P : (ft + 1) * P, :])
                wot_sb.append(w_o)

            # zero-padded per-head Q tiles: qt0 = [Q_h0; 0], qt1 = [0; Q_h1].
            # Score matmuls can then use the full 128-row KT stationary (the
            # zero half contributes nothing), keeping every attention LDWEIGHTS
            # uniform 128-row so the PE hides them behind running matmuls.
            qt0_sb = [persist.tile([P, seq], BF16, name=f"qt0_{i}") for i in range(FT)]
            qt1_sb = [persist.tile([P, seq], BF16, name=f"qt1_{i}") for i in range(FT)]
            for ft in range(FT):
                nc.vector.memset(qt0_sb[ft][64:128, :], 0.0)
                nc.vector.memset(qt1_sb[ft][0:64, :], 0.0)
            kt_sb = [persist.tile([P, seq], BF16, name=f"kt{i}") for i in range(FT)]
            # v2 holds, per 128-col head block: even heads [V_h | ones],
            # odd heads [ones | V_h] -- the ones columns make the PV matmul
            # also produce the softmax denominators on the other 64 rows.
            v2_sb = [persist.tile([P, 2 * DG], BF16, name=f"v{i}") for i in range(KB)]
            for kb in range(KB):
                nc.vector.memset(v2_sb[kb], 1.0)
            onorm = [persist.tile([P, seq], BF16, name=f"onorm{i}") for i in range(FT)]

            # ---------------- phase 1: projections ----------------
            # QT (bias via ACT), then KT, then V -- keeps the PE streaming.
            for c in range(NQC):
                csl = slice(c * QC, (c + 1) * QC)
                for ft in range(FT):
                    fsl = slice(ft * P, (ft + 1) * P)
                    psq = sp_ps.tile([P, QC], F32, name="psq", tag="sp")
                    for k in range(DKB):
                        nc.tensor.matmul(
                            psq,
                            lhsT=wqt_sb[k][:, fsl],
                            rhs=xt_sb[k][:, csl],
                            start=k == 0,
                            stop=k == DKB - 1,
                        )
                    nc.scalar.activation(
                        qt0_sb[ft][0:64, csl], psq[0:64, :], AF.Identity,
                        bias=bq_sb[0:64, ft : ft + 1],
                    )
                    nc.scalar.activation(
                        qt1_sb[ft][64:128, csl], psq[64:128, :], AF.Identity,
                        bias=bq_sb[64:128, ft : ft + 1],
                    )
            for ft in range(FT):
                fsl = slice(ft * P, (ft + 1) * P)
                for c in range(NQC):
                    csl = slice(c * QC, (c + 1) * QC)
                    psk = sp_ps.tile([P, QC], F32, name="psk", tag="sp")
                    for k in range(DKB):
                        nc.tensor.matmul(
                            psk,
                            lhsT=wkt_sb[k][:, fsl],
                            rhs=xt_sb[k][:, csl],
                            start=k == 0,
                            stop=k == DKB - 1,
                        )
                    nc.vector.tensor_copy(kt_sb[ft][:, csl], psk)

            # V: keys on partitions, features on free dim (+ bias via K=1 matmul)
            for kb in range(KB):
                ksl = slice(kb * P, (kb + 1) * P)
                psv = pv_ps.tile([P, DG], F32, name="psv", tag="pvb")
                for k in range(DKB):
                    nc.tensor.matmul(
                        psv,
                        lhsT=xt_sb[k][:, ksl],
                        rhs=wvt_sb[k],
                        start=k == 0,
                        stop=False,
                    )
                nc.tensor.matmul(
                    psv, lhsT=ones_bf, rhs=bv_sb, start=False, stop=True
                )
                # even heads -> cols [256q, 0:64); odd heads -> [256q+192, 256q+256)
                nc.vector.tensor_copy(
                    v2_sb[kb].rearrange("p (q c) -> p q c", c=256)[:, :, 0:64],
                    psv.rearrange("p (q c) -> p q c", c=128)[:, :, 0:64],
                )
                nc.vector.tensor_copy(
                    v2_sb[kb].rearrange("p (q c) -> p q c", c=256)[:, :, 192:256],
                    psv.rearrange("p (q c) -> p q c", c=128)[:, :, 64:128],
                )

            # ---------------- phase 2: attention ----------------
            # Software-pipelined: scores kb | exp kb-1 | PV kb-3, so the PE
            # queue never drains and the exp engines run one step behind with
            # ~1.7us of slack.  The previous stripe's normalization ops are
            # spread across this stripe's steps (keyed by step number) so the
            # DVE never sees a burst that would starve the PE of exp tiles.
            PVLAG = 4
            pending_norm: dict = {}

            def run_pending(step):
                for fn in pending_norm.pop(step, ()):  # noqa: B023
                    fn()

            def flush_norm():
                for st in sorted(pending_norm):
                    for fn in pending_norm[st]:
                        fn()
                pending_norm.clear()

            for pr in range(FT):
                h0c = slice((2 * pr) * P, (2 * pr + 1) * P)
                h1c = slice((2 * pr + 1) * P, (2 * pr + 2) * P)
                for qh in range(NQH):
                    qsl = slice(qh * QH, (qh + 1) * QH)
                    pvb = pv_ps.tile([P, 2 * QH], F32, name="pvb", tag="pvb")
                    sp_l = [None] * KB
                    ep_l = [None] * KB
                    for step in range(KB + PVLAG):
                        kb2 = step - PVLAG
                        if step < KB:
                            kb = step
                            ksl = slice(kb * P, (kb + 1) * P)
                            sp = sp_ps.tile([P, 2 * QH], F32, name="sp", tag="sp")
                            sp_l[kb] = sp
                            nc.tensor.matmul(
                                sp[:, 0:QH],
                                lhsT=kt_sb[pr][:, ksl],
                                rhs=qt0_sb[pr][:, qsl],
                                start=True, stop=True,
                            )
                            if kb2 >= 0:
                                nc.tensor.matmul(
                                    pvb[:, 0:QH], lhsT=v2_sb[kb2][:, h0c],
                                    rhs=ep_l[kb2][:, 0:QH],
                                    start=kb2 == 0, stop=kb2 == KB - 1,
                                )
                            nc.tensor.matmul(
                                sp[:, QH : 2 * QH],
                                lhsT=kt_sb[pr][:, ksl],
                                rhs=qt1_sb[pr][:, qsl],
                                start=True, stop=True,
                            )
                            if kb2 >= 0:
                                nc.tensor.matmul(
                                    pvb[:, QH : 2 * QH], lhsT=v2_sb[kb2][:, h1c],
                                    rhs=ep_l[kb2][:, QH : 2 * QH],
                                    start=kb2 == 0, stop=kb2 == KB - 1,
                                )
                        elif kb2 >= 0:
                            nc.tensor.matmul(
                                pvb[:, 0:QH], lhsT=v2_sb[kb2][:, h0c],
                                rhs=ep_l[kb2][:, 0:QH],
                                start=kb2 == 0, stop=kb2 == KB - 1,
                            )
                            nc.tensor.matmul(
                                pvb[:, QH : 2 * QH], lhsT=v2_sb[kb2][:, h1c],
                                rhs=ep_l[kb2][:, QH : 2 * QH],
                                start=kb2 == 0, stop=kb2 == KB - 1,
                            )
                        if 1 <= step <= KB:
                            kb1 = step - 1
                            ep = e_pool.tile([P, 2 * QH], BF16, name="ep", tag="ep")
                            ep_l[kb1] = ep
                            if kb1 in DVE_KBS:
                                nc.vector.tensor_scalar(
                                    ep.bitcast(I16), sp_l[kb1],
                                    SCHRAUD_A, SCHRAUD_B, MUL, ADD,
                                )
                            else:
                                nc.scalar.activation(ep, sp_l[kb1], AF.Exp)
                            if dbg and pr == 0 and qh == 0 and kb1 in (0, 1):
                                dtmp = m_pool.tile(
                                    [P, 2 * QH], F32, name="dtmp", tag="dtmp"
                                )
                                nc.vector.tensor_copy(dtmp, ep)
                                nc.sync.dma_start(
                                    de_d[:, kb1 * 2 * QH : (kb1 + 1) * 2 * QH], dtmp
                                )
                        run_pending(step)

                    # normalization for this stripe.  pvb is single-
                    # buffered, so it is evacuated to SBUF immediately (one
                    # ACT + one DVE copy, ~1.4us) and the reciprocals /
                    # multiplies run from SBUF, scheduled into the next
                    # stripe's steps.  reciprocal_approx_fast (custom DVE op)
                    # misbehaves at base_partition 64, so both reciprocals run
                    # at base partition 0 after a DMA partition-swap.
                    pvc = m_pool.tile([P, 2 * QH], F32, name="pvc", tag="pvc")
                    nc.scalar.copy(pvc[:, 0:QH], pvb[:, 0:QH])
                    nc.vector.tensor_copy(pvc[:, QH : 2 * QH], pvb[:, QH : 2 * QH])
                    dsw = m_pool.tile([P, QH], F32, name="dsw", tag="dsw")
                    nc.sync.dma_start(dsw[0:64, :], pvc[64:128, 0:QH])
                    rec = m_pool.tile([P, QH], F32, name="rec", tag="rec")
                    rsw = m_pool.tile([P, QH], F32, name="rsw", tag="rsw")

                    def n_rec(pvc=pvc, rec=rec, rsw=rsw):
                        nc.vector.reciprocal_approx_fast(
                            rec[0:64, :], pvc[0:64, QH : 2 * QH]
                        )
                        nc.sync.dma_start(rsw[64:128, :], rec[0:64, :])

                    def n_rsw(dsw=dsw, rsw=rsw):
                        nc.vector.reciprocal_approx_fast(rsw[0:64, :], dsw[0:64, :])

                    def n_mul0(pr=pr, qsl=qsl, pvc=pvc, rsw=rsw):
                        nc.vector.tensor_tensor(
                            onorm[pr][0:64, qsl], pvc[0:64, 0:QH],
                            rsw[0:64, :], MUL,
                        )

                    def n_mul1(pr=pr, qsl=qsl, pvc=pvc, rsw=rsw):
                        nc.vector.tensor_tensor(
                            onorm[pr][64:128, qsl], pvc[64:128, QH : 2 * QH],
                            rsw[64:128, :], MUL,
                        )

                    pending_norm = {2: [n_rec], 6: [n_rsw],
                                    8: [n_mul0], 10: [n_mul1]}
                    if dbg and pr == 0 and qh == 0:
                        dtmp2 = m_pool.tile([P, 2 * QH], F32, name="dtmp2", tag="dtmp")
                        nc.vector.tensor_copy(dtmp2, pvb)
                        nc.sync.dma_start(dpv_d[:, :], dtmp2)

            flush_norm()

            if dbg:
                with tc.tile_pool(name="dbgp", bufs=2) as dbg_pool:
                    for ft in range(FT):
                        fs = slice(ft * P, (ft + 1) * P)
                        dqs = dbg_pool.tile([P, seq], F32, name="dqs", tag="dbg")
                        nc.vector.tensor_copy(dqs[0:64, :], qt0_sb[ft][0:64, :])
                        nc.vector.tensor_copy(dqs[64:128, :], qt1_sb[ft][64:128, :])
                        nc.sync.dma_start(dq_d[fs, :], dqs)
                        dks = dbg_pool.tile([P, seq], F32, name="dks", tag="dbg")
                        nc.vector.tensor_copy(dks, kt_sb[ft])
                        nc.sync.dma_start(dk_d[fs, :], dks)
                        dns = dbg_pool.tile([P, seq], F32, name="dns", tag="dbg")
                        nc.vector.tensor_copy(dns, onorm[ft])
                        nc.sync.dma_start(dn_d[fs, :], dns)
                    for kb in range(KB):
                        ks = slice(kb * P, (kb + 1) * P)
                        dvs = dbg_pool.tile([P, 2 * DG], F32, name="dvs", tag="dbg")
                        nc.vector.tensor_copy(dvs, v2_sb[kb])
                        nc.sync.dma_start(dv_d[ks, :], dvs)

            # ---------------- phase 3: output projection ----------------
            with tc.tile_pool(name="osb", bufs=4) as o_sb_pool:
                for dt in range(DT):
                    dsl = slice(dt * P, (dt + 1) * P)
                    for c in range(NQC):
                        csl = slice(c * QC, (c + 1) * QC)
                        pso = sp_ps.tile([P, QC], F32, name="pso", tag="sp")
                        for ft in range(FT):
                            nc.tensor.matmul(
                                pso,
                                lhsT=wot_sb[ft][:, dsl],
                                rhs=onorm[ft][:, csl],
                                start=ft == 0,
                                stop=ft == FT - 1,
                            )
                        o_sb = o_sb_pool.tile([P, QC], BF16, name="o_sb", tag="osb")
                        if (dt * NQC + c) % 2 == 0:
                            nc.vector.tensor_copy(o_sb, pso)
                        else:
                            nc.scalar.copy(o_sb, pso)
                        nc.sync.dma_start(out_d[dsl, csl], o_sb)

    return nc


_CACHE: dict = {}


def _get_nc(seq: int = S) -> bass.Bass:
    key = f"nc{seq}"
    if key not in _CACHE:
        nc = build_attention_nc(seq)
        nc.finalize()  # runs Bacc.compile(): reg alloc + wait legalization
        _CACHE[key] = nc
    return _CACHE[key]


def make_in_maps(x, Wq, bq, Wk, Wv, bv, Wo, seq: int = S):
    bf = ml_dtypes.bfloat16
    scale = 1.0 / math.sqrt(DK)
    x = np.asarray(x, np.float32)
    Wq = np.asarray(Wq, np.float32)
    bq = np.asarray(bq, np.float32)
    Wk = np.asarray(Wk, np.float32)
    Wv = np.asarray(Wv, np.float32)
    bv = np.asarray(bv, np.float32)
    Wo = np.asarray(Wo, np.float32)
    in_maps = []
    for core in range(NCORES):
        b, g = divmod(core, GROUPS)
        gsl = slice(g * DG, (g + 1) * DG)
        in_maps.append(
            {
                "xt": np.ascontiguousarray(x[b, :seq, :].T).astype(bf),
                "wqt": np.ascontiguousarray((Wq[gsl, :] * scale).T).astype(bf),
                "wkt": np.ascontiguousarray(Wk[gsl, :].T).astype(bf),
                "wvt": np.ascontiguousarray(Wv[gsl, :].T).astype(bf),
                "wot": np.ascontiguousarray(Wo[:, gsl].T).astype(bf),
                "bqs": np.ascontiguousarray(
                    (bq[gsl] * scale).astype(np.float32).reshape(FT, P).T
                ),
                "bvr": bv[gsl].astype(bf).reshape(1, DG),
            }
        )
    return in_maps


def run_device(in_maps, seq: int = S, trace: bool = False):
    nc = _get_nc(seq)
    return run_bass_kernel_spmd(nc, in_maps, list(range(NCORES)), trace=trace)


def kernel(x, Wq, bq, Wk, bk, Wv, bv, Wo, bo):
    in_maps = make_in_maps(x, Wq, bq, Wk, Wv, bv, Wo)
    res = run_device(in_maps).results
    bo = np.asarray(bo, np.float32)
    out = np.empty((B, S, D), np.float32)
    for b in range(B):
        acc = res[2 * b]["out"].astype(np.float32) + res[2 * b + 1]["out"].astype(
            np.float32
        )
        out[b] = acc.T + bo[None, :]
    return out


# revision 19
# speedup vs baseline: 1.2685x; 1.0389x over previous
"""Multi-head self-attention on 8 Trainium2 NeuronCores.

Problem: B=4, S=2048, D=1024, H=16 heads (dk=64), torch-Linear style
projections (y = x @ W.T + b), softmax attention, output projection.

Sharding: 8 cores = 4 batches x 2 head-groups (8 heads each).  Each core
computes, for its (batch b, group g):
    QT = (Wq_g/sqrt(dk)) @ x_b.T + bq_g/sqrt(dk)   [512, S]  (features on partitions)
    KT = Wk_g @ x_b.T                              [512, S]  (bk dropped: it only
                                                    shifts scores uniformly per
                                                    query and cancels in softmax)
    V  = x_b @ Wv_g.T + bv_g                       [S, 512]  (keys on partitions)
    per head pair pr (even head h0, odd h1), query stripe of 512, key block kb:
      sp[kb] = [scoresT_h0 | scoresT_h1]  (one 2-bank PSUM tile [128, 1024])
      ep[kb] = exp(sp[kb])  -- computed on ScalarE (exact spline exp) for 10 of
               16 key blocks and on VectorE for 6 of 16 via a one-instruction
               Schraudolph approximation: int16(round(s*184.665 + B)) bit-cast
               to bf16 == 2^(s*log2 e) with ~2% rms error.  Splitting exp
               across both engines keeps the TensorE (the bottleneck) saturated
               so the HAM clock gate stays at 2.4 GHz.
      pvb accumulates [V_h0|ones]^T ep_h0 and [ones|V_h1]^T ep_h1, yielding
      outputs and softmax denominators (ones-columns trick).
    normalization: reciprocal of the denominators (DVE approx), partition-swap
    of the two heads' reciprocals via SBUF->SBUF DMA, two tensor-tensor mults.
    partialT = Wo_g @ onorm     [1024, S]
Host sums the two group partials per batch, transposes, and adds bo.

Device dtypes: bf16 matmul operands, f32 PSUM; exp split ScalarE/VectorE.
"""

import math

import numpy as np
import ml_dtypes

import concourse.bass as bass
import concourse.bacc as bacc_mod
import concourse.mybir as mybir
import concourse.tile as tile
from concourse.bass_utils import run_bass_kernel_spmd

BF16 = mybir.dt.bfloat16
F32 = mybir.dt.float32
I16 = mybir.dt.int16
AF = mybir.ActivationFunctionType
MUL = mybir.AluOpType.mult
ADD = mybir.AluOpType.add

B, S, D, H = 4, 2048, 1024, 16
DK = D // H  # 64
NCORES = 8
GROUPS = 2  # tensor-parallel head groups
DG = D // GROUPS  # 512 features per group
P = 128
FT = DG // P  # 4 feature tiles per group == head pairs

# Schraudolph exp constants (bf16 bit trick): bits = s * A + BSCH, viewed bf16.
SCHRAUD_A = 128.0 / math.log(2.0)
SCHRAUD_B = 16256.0 - 7.25

# key blocks whose exp runs on VectorE (the rest go to ScalarE)
DVE_KBS = frozenset((1, 3, 5, 7, 9, 11, 13, 15))


def build_attention_nc(seq: int = S, dbg: bool = False) -> bass.Bass:
    KB = seq // P  # 16 key blocks
    DKB = D // P  # 8 contraction blocks for projections
    QH = min(512, seq)  # query stripe == PSUM bank width in f32
    NQH = seq // QH
    QC = min(512, QH)
    NQC = seq // QC
    DT = D // P

    nc = bacc_mod.Bacc("TRN2", num_devices=NCORES)
    xt_d = nc.declare_dram_parameter("xt", [D, seq], BF16, isOutput=False)
    wqt_d = nc.declare_dram_parameter("wqt", [D, DG], BF16, isOutput=False)
    wkt_d = nc.declare_dram_parameter("wkt", [D, DG], BF16, isOutput=False)
    wvt_d = nc.declare_dram_parameter("wvt", [D, DG], BF16, isOutput=False)
    wot_d = nc.declare_dram_parameter("wot", [DG, D], BF16, isOutput=False)
    bq_d = nc.declare_dram_parameter("bqs", [P, FT], F32, isOutput=False)
    bv_d = nc.declare_dram_parameter("bvr", [1, DG], BF16, isOutput=False)
    out_d = nc.declare_dram_parameter("out", [D, seq], BF16, isOutput=True)
    if dbg:
        dq_d = nc.declare_dram_parameter("dq", [DG, seq], F32, isOutput=True)
        dk_d = nc.declare_dram_parameter("dk", [DG, seq], F32, isOutput=True)
        dv_d = nc.declare_dram_parameter("dv", [P * KB, 2 * DG], F32, isOutput=True)
        de_d = nc.declare_dram_parameter("de", [P, 4 * 512], F32, isOutput=True)
        dpv_d = nc.declare_dram_parameter("dpv", [P, 2 * 512], F32, isOutput=True)
        dr_d = nc.declare_dram_parameter("dr", [P, 2 * 512], F32, isOutput=True)
        dn_d = nc.declare_dram_parameter("dn", [DG, seq], F32, isOutput=True)

    with tile.TileContext(nc) as tc:
        with (
            tc.tile_pool(name="persist", bufs=1) as persist,
            tc.tile_pool(name="sp_ps", bufs=3, space="PSUM") as sp_ps,
            tc.tile_pool(name="pv_ps", bufs=1, space="PSUM") as pv_ps,
            tc.tile_pool(name="epool", bufs=6) as e_pool,
            tc.tile_pool(name="mpool", bufs=2) as m_pool,
        ):
            ones_bf = persist.tile([1, P], BF16, name="ones_bf")
            nc.vector.memset(ones_bf, 1.0)
            bq_sb = persist.tile([P, FT], F32, name="bq_sb")
            nc.scalar.dma_start(bq_sb, bq_d[:, :])
            bv_sb = persist.tile([1, DG], BF16, name="bv_sb")
            nc.scalar.dma_start(bv_sb, bv_d[:, :])

            # input DMAs, ordered so QT can start as early as possible
            xt_sb = []
            wqt_sb = []
            wkt_sb = []
            wvt_sb = []
            for i in range(DKB):
                wq_i = persist.tile([P, DG], BF16, name=f"wqts{i}")
                nc.scalar.dma_start(wq_i, wqt_d[i * P : (i + 1) * P, :])
                wqt_sb.append(wq_i)
                xti = persist.tile([P, seq], BF16, name=f"xts{i}")
                # first query-column chunk of every k-block lands first so the
                # QT matmuls can start ~4x earlier
                nc.sync.dma_start(
                    xti[:, 0:QC], xt_d[i * P : (i + 1) * P, 0:QC]
                )
                xt_sb.append(xti)
            for c in range(1, NQC):
                csl = slice(c * QC, (c + 1) * QC)
                for i in range(DKB):
                    nc.sync.dma_start(
                        xt_sb[i][:, csl], xt_d[i * P : (i + 1) * P, csl]
                    )
            for i in range(DKB):
                wk_i = persist.tile([P, DG], BF16, name=f"wkts{i}")
                nc.sync.dma_start(wk_i, wkt_d[i * P : (i + 1) * P, :])
                wkt_sb.append(wk_i)
            for i in range(DKB):
                wv_i = persist.tile([P, DG], BF16, name=f"wvts{i}")
                nc.sync.dma_start(wv_i, wvt_d[i * P : (i + 1) * P, :])
                wvt_sb.append(wv_i)
            wot_sb = []
            for ft in range(FT):
                w_o = persist.tile([P, D], BF16, name=f"wot{ft}")
                nc.sync.dma_start(w_o, wot_d[ft * P : (ft + 1) * P, :])
                wot_sb.append(w_o)

            # zero-padded per-head Q tiles: qt0 = [Q_h0; 0], qt1 = [0; Q_h1].
            # Score matmuls can then use the full 128-row KT stationary (the
            # zero half contributes nothing), keeping every attention LDWEIGHTS
            # uniform 128-row so the PE hides them behind running matmuls.
            qt0_sb = [persist.tile([P, seq], BF16, name=f"qt0_{i}") for i in range(FT)]
            qt1_sb = [persist.tile([P, seq], BF16, name=f"qt1_{i}") for i in range(FT)]
            for ft in range(FT):
                nc.vector.memset(qt0_sb[ft][64:128, :], 0.0)
                nc.vector.memset(qt1_sb[ft][0:64, :], 0.0)
            kt_sb = [persist.tile([P, seq], BF16, name=f"kt{i}") for i in range(FT)]
            # v2 holds, per 128-col head block: even heads [V_h | ones],
            # odd heads [ones | V_h] -- the ones columns make the PV matmul
            # also produce the softmax denominators on the other 64 rows.
            v2_sb = [persist.tile([P, 2 * DG], BF16, name=f"v{i}") for i in range(KB)]
            for kb in range(KB):
                nc.vector.memset(v2_sb[kb], 1.0)
            onorm = [persist.tile([P, seq], BF16, name=f"onorm{i}") for i in range(FT)]

            # ---------------- phase 1: projections ----------------
            # QT (bias via ACT), then KT, then V -- keeps the PE streaming.
            for c in range(NQC):
                csl = slice(c * QC, (c + 1) * QC)
                for ft in range(FT):
                    fsl = slice(ft * P, (ft + 1) * P)
                    psq = sp_ps.tile([P, QC], F32, name="psq", tag="sp")
                    for k in range(DKB):
                        nc.tensor.matmul(
                            psq,
                            lhsT=wqt_sb[k][:, fsl],
                            rhs=xt_sb[k][:, csl],
                            start=k == 0,
                            stop=k == DKB - 1,
                        )
                    nc.scalar.activation(
                        qt0_sb[ft][0:64, csl], psq[0:64, :], AF.Identity,
                        bias=bq_sb[0:64, ft : ft + 1],
                    )
                    nc.scalar.activation(
                        qt1_sb[ft][64:128, csl], psq[64:128, :], AF.Identity,
                        bias=bq_sb[64:128, ft : ft + 1],
                    )
            for ft in range(FT):
                fsl = slice(ft * P, (ft + 1) * P)
                for c in range(NQC):
                    csl = slice(c * QC, (c + 1) * QC)
                    psk = sp_ps.tile([P, QC], F32, name="psk", tag="sp")
                    for k in range(DKB):
                        nc.tensor.matmul(
                            psk,
                            lhsT=wkt_sb[k][:, fsl],
                            rhs=xt_sb[k][:, csl],
                            start=k == 0,
                            stop=k == DKB - 1,
                        )
                    nc.vector.tensor_copy(kt_sb[ft][:, csl], psk)

            # V: keys on partitions, features on free dim (+ bias via K=1 matmul)
            for kb in range(KB):
                ksl = slice(kb * P, (kb + 1) * P)
                psv = sp_ps.tile([P, DG], F32, name="psv", tag="sp")
                for k in range(DKB):
                    nc.tensor.matmul(
                        psv,
                        lhsT=xt_sb[k][:, ksl],
                        rhs=wvt_sb[k],
                        start=k == 0,
                        stop=False,
                    )
                nc.tensor.matmul(
                    psv, lhsT=ones_bf, rhs=bv_sb, start=False, stop=True
                )
                # even heads -> cols [256q, 0:64); odd heads -> [256q+192, 256q+256)
                nc.vector.tensor_copy(
                    v2_sb[kb].rearrange("p (q c) -> p q c", c=256)[:, :, 0:64],
                    psv.rearrange("p (q c) -> p q c", c=128)[:, :, 0:64],
                )
                nc.vector.tensor_copy(
                    v2_sb[kb].rearrange("p (q c) -> p q c", c=256)[:, :, 192:256],
                    psv.rearrange("p (q c) -> p q c", c=128)[:, :, 64:128],
                )

            # ---------------- phase 2: attention ----------------
            # Software-pipelined: scores kb | exp kb-1 | PV kb-3, so the PE
            # queue never drains and the exp engines run one step behind with
            # ~1.7us of slack.  The previous stripe's normalization ops are
            # spread across this stripe's steps (keyed by step number) so the
            # DVE never sees a burst that would starve the PE of exp tiles.
            PVLAG = 4
            pending_norm: dict = {}

            def run_pending(step):
                for fn in pending_norm.pop(step, ()):  # noqa: B023
                    fn()

            def flush_norm():
                for st in sorted(pending_norm):
                    for fn in pending_norm[st]:
                        fn()
                pending_norm.clear()

            for pr in range(FT):
                h0c = slice((2 * pr) * P, (2 * pr + 1) * P)
                h1c = slice((2 * pr + 1) * P, (2 * pr + 2) * P)
                for qh in range(NQH):
                    qsl = slice(qh * QH, (qh + 1) * QH)
                    pvb = pv_ps.tile([P, 2 * QH], F32, name="pvb", tag="pvb")
                    sp_l = [None] * KB
                    ep_l = [None] * KB
                    for step in range(KB + PVLAG):
                        kb2 = step - PVLAG
                        if step < KB:
                            kb = step
                            ksl = slice(kb * P, (kb + 1) * P)
                            sp = sp_ps.tile([P, 2 * QH], F32, name="sp", tag="sp")
                            sp_l[kb] = sp
                            nc.tensor.matmul(
                                sp[:, 0:QH],
                                lhsT=kt_sb[pr][:, ksl],
                                rhs=qt0_sb[pr][:, qsl],
                                start=True, stop=True,
                            )
                            if kb2 >= 0:
                                nc.tensor.matmul(
                                    pvb[:, 0:QH], lhsT=v2_sb[kb2][:, h0c],
                                    rhs=ep_l[kb2][:, 0:QH],
                                    start=kb2 == 0, stop=kb2 == KB - 1,
                                )
                            nc.tensor.matmul(
                                sp[:, QH : 2 * QH],
                                lhsT=kt_sb[pr][:, ksl],
                                rhs=qt1_sb[pr][:, qsl],
                                start=True, stop=True,
                            )
                            if kb2 >= 0:
                                nc.tensor.matmul(
                                    pvb[:, QH : 2 * QH], lhsT=v2_sb[kb2][:, h1c],
                                    rhs=ep_l[kb2][:, QH : 2 * QH],
                                    start=kb2 == 0, stop=kb2 == KB - 1,
                                )
                        elif kb2 >= 0:
                            nc.tensor.matmul(
                                pvb[:, 0:QH], lhsT=v2_sb[kb2][:, h0c],
                                rhs=ep_l[kb2][:, 0:QH],
                                start=kb2 == 0, stop=kb2 == KB - 1,
                            )
                            nc.tensor.matmul(
                                pvb[:, QH : 2 * QH], lhsT=v2_sb[kb2][:, h1c],
                                rhs=ep_l[kb2][:, QH : 2 * QH],
                                start=kb2 == 0, stop=kb2 == KB - 1,
                            )
                        if 1 <= step <= KB:
                            kb1 = step - 1
                            ep = e_pool.tile([P, 2 * QH], BF16, name="ep", tag="ep")
                            ep_l[kb1] = ep
                            if kb1 in DVE_KBS:
                                nc.vector.tensor_scalar(
                                    ep.bitcast(I16), sp_l[kb1],
                                    SCHRAUD_A, SCHRAUD_B, MUL, ADD,
                                )
                            else:
                                nc.scalar.activation(ep, sp_l[kb1], AF.Exp)
                            if dbg and pr == 0 and qh == 0 and kb1 in (0, 1):
                                dtmp = m_pool.tile(
                                    [P, 2 * QH], F32, name="dtmp", tag="dtmp"
                                )
                                nc.vector.tensor_copy(dtmp, ep)
                                nc.sync.dma_start(
                                    de_d[:, kb1 * 2 * QH : (kb1 + 1) * 2 * QH], dtmp
                                )
                        run_pending(step)

                    # normalization for this stripe.  pvb is single-
                    # buffered, so it is evacuated to SBUF immediately (one
                    # ACT + one DVE copy, ~1.4us) and the reciprocals /
                    # multiplies run from SBUF, scheduled into the next
                    # stripe's steps.  reciprocal_approx_fast (custom DVE op)
                    # misbehaves at base_partition 64, so both reciprocals run
                    # at base partition 0 after a DMA partition-swap.
                    pvc = m_pool.tile([P, 2 * QH], F32, name="pvc", tag="pvc")
                    nc.scalar.copy(pvc[:, 0:QH], pvb[:, 0:QH])
                    nc.vector.tensor_copy(pvc[:, QH : 2 * QH], pvb[:, QH : 2 * QH])
                    dsw = m_pool.tile([P, QH], F32, name="dsw", tag="dsw")
                    nc.sync.dma_start(dsw[0:64, :], pvc[64:128, 0:QH])
                    rec = m_pool.tile([P, QH], F32, name="rec", tag="rec")
                    rsw = m_pool.tile([P, QH], F32, name="rsw", tag="rsw")

                    def n_rec(pvc=pvc, rec=rec, rsw=rsw):
                        nc.vector.reciprocal_approx_fast(
                            rec[0:64, :], pvc[0:64, QH : 2 * QH]
                        )
                        nc.sync.dma_start(rsw[64:128, :], rec[0:64, :])

                    def n_rsw(dsw=dsw, rsw=rsw):
                        nc.vector.reciprocal_approx_fast(rsw[0:64, :], dsw[0:64, :])

                    def n_mul0(pr=pr, qsl=qsl, pvc=pvc, rsw=rsw):
                        nc.vector.tensor_tensor(
                            onorm[pr][0:64, qsl], pvc[0:64, 0:QH],
                            rsw[0:64, :], MUL,
                        )

                    def n_mul1(pr=pr, qsl=qsl, pvc=pvc, rsw=rsw):
                        nc.vector.tensor_tensor(
                            onorm[pr][64:128, qsl], pvc[64:128, QH : 2 * QH],
                            rsw[64:128, :], MUL,
                        )

                    pending_norm = {2: [n_rec], 6: [n_rsw],
                                    8: [n_mul0], 10: [n_mul1]}
                    if dbg and pr == 0 and qh == 0:
                        dtmp2 = m_pool.tile([P, 2 * QH], F32, name="dtmp2", tag="dtmp")
                        nc.vector.tensor_copy(dtmp2, pvb)
                        nc.sync.dma_start(dpv_d[:, :], dtmp2)

            flush_norm()

            if dbg:
                with tc.tile_pool(name="dbgp", bufs=2) as dbg_pool:
                    for ft in range(FT):
                        fs = slice(ft * P, (ft + 1) * P)
                        dqs = dbg_pool.tile([P, seq], F32, name="dqs", tag="dbg")
                        nc.vector.tensor_copy(dqs[0:64, :], qt0_sb[ft][0:64, :])
                        nc.vector.tensor_copy(dqs[64:128, :], qt1_sb[ft][64:128, :])
                        nc.sync.dma_start(dq_d[fs, :], dqs)
                        dks = dbg_pool.tile([P, seq], F32, name="dks", tag="dbg")
                        nc.vector.tensor_copy(dks, kt_sb[ft])
                        nc.sync.dma_start(dk_d[fs, :], dks)
                        dns = dbg_pool.tile([P, seq], F32, name="dns", tag="dbg")
                        nc.vector.tensor_copy(dns, onorm[ft])
                        nc.sync.dma_start(dn_d[fs, :], dns)
                    for kb in range(KB):
                        ks = slice(kb * P, (kb + 1) * P)
                        dvs = dbg_pool.tile([P, 2 * DG], F32, name="dvs", tag="dbg")
                        nc.vector.tensor_copy(dvs, v2_sb[kb])
                        nc.sync.dma_start(dv_d[ks, :], dvs)

            # ---------------- phase 3: output projection ----------------
            with tc.tile_pool(name="osb", bufs=4) as o_sb_pool:
                for dt in range(DT):
                    dsl = slice(dt * P, (dt + 1) * P)
                    for c in range(NQC):
                        csl = slice(c * QC, (c + 1) * QC)
                        pso = sp_ps.tile([P, QC], F32, name="pso", tag="sp")
                        for ft in range(FT):
                            nc.tensor.matmul(
                                pso,
                                lhsT=wot_sb[ft][:, dsl],
                                rhs=onorm[ft][:, csl],
                                start=ft == 0,
                                stop=ft == FT - 1,
                            )
                        o_sb = o_sb_pool.tile([P, QC], BF16, name="o_sb", tag="osb")
                        if (dt * NQC + c) % 2 == 0:
                            nc.vector.tensor_copy(o_sb, pso)
                        else:
                            nc.scalar.copy(o_sb, pso)
                        (nc.sync if c % 2 else nc.scalar).dma_start(
                            out_d[dsl, csl], o_sb
                        )

    return nc


_CACHE: dict = {}


def _get_nc(seq: int = S) -> bass.Bass:
    key = f"nc{seq}"
    if key not in _CACHE:
        nc = build_attention_nc(seq)
        nc.finalize()  # runs Bacc.compile(): reg alloc + wait legalization
        _CACHE[key] = nc
    return _CACHE[key]


def make_in_maps(x, Wq, bq, Wk, Wv, bv, Wo, seq: int = S):
    bf = ml_dtypes.bfloat16
    scale = 1.0 / math.sqrt(DK)
    x = np.asarray(x, np.float32)
    Wq = np.asarray(Wq, np.float32)
    bq = np.asarray(bq, np.float32)
    Wk = np.asarray(Wk, np.float32)
    Wv = np.asarray(Wv, np.float32)
    bv = np.asarray(bv, np.float32)
    Wo = np.asarray(Wo, np.float32)
    in_maps = []
    for core in range(NCORES):
        b, g = divmod(core, GROUPS)
        gsl = slice(g * DG, (g + 1) * DG)
        in_maps.append(
            {
                "xt": np.ascontiguousarray(x[b, :seq, :].T).astype(bf),
                "wqt": np.ascontiguousarray((Wq[gsl, :] * scale).T).astype(bf),
                "wkt": np.ascontiguousarray(Wk[gsl, :].T).astype(bf),
                "wvt": np.ascontiguousarray(Wv[gsl, :].T).astype(bf),
                "wot": np.ascontiguousarray(Wo[:, gsl].T).astype(bf),
                "bqs": np.ascontiguousarray(
                    (bq[gsl] * scale).astype(np.float32).reshape(FT, P).T
                ),
                "bvr": bv[gsl].astype(bf).reshape(1, DG),
            }
        )
    return in_maps


def run_device(in_maps, seq: int = S, trace: bool = False):
    nc = _get_nc(seq)
    return run_bass_kernel_spmd(nc, in_maps, list(range(NCORES)), trace=trace)


def kernel(x, Wq, bq, Wk, bk, Wv, bv, Wo, bo):
    in_maps = make_in_maps(x, Wq, bq, Wk, Wv, bv, Wo)
    res = run_device(in_maps).results
    bo = np.asarray(bo, np.float32)
    out = np.empty((B, S, D), np.float32)
    for b in range(B):
        acc = res[2 * b]["out"].astype(np.float32) + res[2 * b + 1]["out"].astype(
            np.float32
        )
        out[b] = acc.T + bo[None, :]
    return out
